# revision 16
# baseline (speedup 1.0000x reference)
"""Trainium2 Bass kernel for the GNN message-passing model.

Strategy: pure data-parallel over batch (B=16 -> 2 batches per core, 8 cores,
no cross-core communication).

v2 changes over the 316us baseline (which was front-end latency-bound with a
29us startup, a ~120us under-pipelined positional front-end, and HAM
oscillation from PE gaps):
  * Adjacency DRAM layout is pre-staged on host as [8 stripes, 128, 32, 512]
    so each 2MiB stripe DMA is fully contiguous on both sides (128 x 16KiB
    descriptors instead of 4096 x 512B software packets).
  * All shared weights are packed on host into two tensors (cf32/cb16) and
    land in SBUF via two contiguous DMAs instead of ~20 small ones.
  * The sin/cos range reduction is restructured: the frequency-expansion
    matmul gets an extra "ones" input row so it directly emits both sin args
    (rows 0:62) and cos args (rows 62:124, = t+0.25); round(t) is ONE fused
    DVE tensor_scalar ((t+M)-M), frac is one tensor_sub. 2 DVE ops per chunk
    instead of 7.
  * Mesh is DMA'd once ([7,4096] incl. ones row); per-chunk mesh slices feed
    the PE directly and pein raw-mesh rows are tiny gpsimd SBUF copies, so
    the scalar queue carries no per-chunk DMAs.
  * Front-end per-chunk elementwise work is split evenly scalar/DVE.
  * Adjacency matmul phases use ib-groups of 2 with drains alternating
    between scalar and DVE; last layer drains are ONE fused [114,512]
    tensor_reduce per ib.
  * Big memsets dropped (fcst/fcutT/mx) except fcutT rows 42:64 which must
    stay zero (they flow through the PE transpose where NaN*0 would
    contaminate); pein zero rows kept.

Weight-only folds done on host (pure parameter preprocessing):
  W3fold = pw3 @ gw0[100:200]   (positional-MLP last layer folded into gw0)
  t4     = emb @ gw0[200:300]   (embedding table folded into gw0)
  pb3f   = pb3 @ gw0[100:200]   (bias fold)
mask_idx is re-encoded as a one-hot (4 classes) so the embedding lookup
becomes a K=4 matmul accumulated into the same PSUM as the layer-0 matmul.
Batch 1's feature order is PERMUTED (cut features moved to rows 64:106/114)
so both batches pack into one adjacency-matmul stationary tile.
"""

import numpy as np
import ml_dtypes

import concourse.bass as bass
import concourse.mybir as mybir
import concourse.tile as tile
from concourse.masks import make_identity
from concourse.bass_utils import run_bass_kernel_spmd

F32 = mybir.dt.float32
BF16 = mybir.dt.bfloat16
FP8 = mybir.dt.float8e4
AF = mybir.ActivationFunctionType
ALU = mybir.AluOpType
BF = ml_dtypes.bfloat16
E4 = ml_dtypes.float8_e4m3

B, N, BC = 16, 4096, 2          # batches, nodes, batches per core
NCORES = 8
NB = N // 512                   # 8 column blocks of 512
NQ = N // 128                   # 32 contraction sub-blocks of 128
MAGIC = float(1.5 * 2 ** 23)    # fp32 round-to-nearest magic constant
TWO_PI = float(2.0 * np.pi)
SA = float(2.0 ** 19)           # adjacency fp8 scale
SF = 64.0                       # cut-feature fp8 scale
INV = float(1.0 / (SA * SF))    # undo scale after the adjacency matmul
MST = 128                       # fcst stationary slot stride (bytes, fp8)
DR = mybir.MatmulPerfMode.DoubleRow

run_kwargs = {}                 # test.py may inject trace kwargs here

# ---- packed-constant column offsets ----
# cf32 [128, CF_W] fp32 segments: (col, width, rows)
CF = {
    "aw1a": (0, 128, 50), "aw1b": (128, 72, 50),
    "aw2a": (200, 100, 128), "aw2b": (300, 100, 72),
    "aw3": (400, 100, 100),
    "gw0L": (500, 128, 100), "gw0Lp": (628, 128, 100),
    "biases": (756, 12, 128),
    "selfreq": (768, 126, 36),
}
CF_W = 894
# cb16 [128, CB_W] bf16 segments
CB = {
    "pw1": (0, 25, 67), "pw2": (25, 50, 25),
    "w3f": (75, 128, 50), "w3fp": (203, 128, 50),
    "t4": (331, 128, 4), "t4p": (459, 128, 4),    # rows 64:68
    "gw1": (587, 128, 128), "gw1p": (715, 128, 128),
    "gw2": (843, 128, 128), "gw2p": (971, 128, 128),
    "gw3": (1099, 50, 128), "gw3p": (1149, 50, 128),
}
CB_EARLY = 587                  # cols 0:587 are needed by the front-end
CB_W = 1199


def split_excess_waits(nc, max_waits=1):
    """Walrus codegen on this image rejects >1 sem wait per instruction;
    move excess waits onto preceding same-engine no-ops."""
    n_split = 0
    for fn in nc.m.functions:
        for blk in fn.blocks:
            insts = list(blk.instructions)
            out = []
            changed = False
            for inst in insts:
                si = getattr(inst, "sync_info", None)
                if si is not None and len(si.on_wait) > max_waits:
                    waits = list(si.on_wait)
                    chunks = [waits[i:i + max_waits]
                              for i in range(0, len(waits), max_waits)]
                    for ci, ch in enumerate(chunks[:-1]):
                        nop = mybir.InstNoOp(
                            name=f"{inst.name}-wsplit-{ci}", ins=[], outs=[])
                        nop.engine = inst.engine
                        nop.sync_info = mybir.SyncInfo(on_wait=ch, on_update=[])
                        out.append(nop)
                        n_split += 1
                    inst.sync_info = mybir.SyncInfo(
                        on_wait=chunks[-1], on_update=list(si.on_update))
                    changed = True
                out.append(inst)
            if changed:
                blk.instructions = out
    return n_split


def _param(nc, name, shape, dt):
    return nc.declare_dram_parameter(name, list(shape), dt, isOutput=False)


def build_bass(split=True):
    nc = bass.Bass()

    adjq = _param(nc, "adjq", [NB, 128, NQ, 512], FP8)
    mesh7d = _param(nc, "mesh7d", [36, N], F32)
    onehotd = _param(nc, "onehotd", [4, BC * N], BF16)
    maskTd = _param(nc, "maskTd", [50, BC], F32)
    cf32d = _param(nc, "cf32d", [128, CF_W], F32)
    cb16d = _param(nc, "cb16d", [128, CB_W], BF16)
    outd = nc.declare_dram_parameter("outd", [128, 1], F32, isOutput=True)

    with tile.TileContext(nc) as tc:
        _emit(nc, tc, locals())
    if split:
        split_excess_waits(nc)
    return nc


def _emit(nc, tc, d):
    import contextlib
    ctx = contextlib.ExitStack()
    adjq, onehotd, maskTd = d["adjq"], d["onehotd"], d["maskTd"]
    outd = d["outd"]

    cpool = ctx.enter_context(tc.tile_pool(name="consts", bufs=1))
    resp = ctx.enter_context(tc.tile_pool(name="resadj", bufs=1))
    actp = ctx.enter_context(tc.tile_pool(name="acts", bufs=1))
    smallp = ctx.enter_context(tc.tile_pool(name="small", bufs=2))
    dvep = ctx.enter_context(tc.tile_pool(name="dvework", bufs=2))

    ps_misc = ctx.enter_context(tc.tile_pool(name="psmisc", bufs=2, space="PSUM"))
    ps_tp = ctx.enter_context(tc.tile_pool(name="pstp", bufs=2, space="PSUM"))
    ps_left = ctx.enter_context(tc.tile_pool(name="psleft", bufs=4, space="PSUM"))

    # ---------------- constant + input DMAs ----------------
    # boot DMAs split over the scalar and sync rings so the DMA engines'
    # per-ring round-robin drains them quickly; bulk adjacency rides gpsimd.
    maskT = cpool.tile([50, BC], F32, tag="maskT", name="maskT")
    nc.scalar.dma_start(out=maskT[:], in_=maskTd[:])
    cf = cpool.tile([128, CF_W], F32, tag="cf", name="cf")
    nc.scalar.dma_start(out=cf[:, 756:], in_=d["cf32d"][:, 756:])
    nc.scalar.dma_start(out=cf[:, 0:756], in_=d["cf32d"][:, 0:756])
    cb = cpool.tile([128, CB_W], BF16, tag="cb", name="cb")

    def vf(key):
        c0, w, r = CF[key]
        return cf[0:r, c0:c0 + w]

    def vb(key, r0=0):
        c0, w, r = CB[key]
        return cb[r0:r0 + r, c0:c0 + w]

    # union tile: mesh (+ones row) lives on partitions 0:7 as f32, the
    # one-hot mask embedding input on partitions 64:68 as bf16 -- disjoint
    # partition rows sharing one 16KiB byte range.
    u7 = cpool.tile([128, N], F32, tag="u7", name="u7")
    onehot = u7[:].bitcast(BF16)
    for mq in range(4):
        ms = slice(mq * 1024, (mq + 1) * 1024)
        nc.sync.dma_start(out=u7[0:36, ms], in_=d["mesh7d"][:, ms])
    nc.sync.dma_start(out=cb[:, 0:CB_EARLY], in_=d["cb16d"][:, 0:CB_EARLY])
    nc.sync.dma_start(out=onehot[64:68, :], in_=onehotd[:])
    nc.sync.dma_start(out=cb[:, CB_EARLY:], in_=d["cb16d"][:, CB_EARLY:])

    # adjacency stripes all ride the gpsimd ring in consumption order,
    # GATED behind the critical boot DMAs (cf32 + mesh): the DMA engines
    # round-robin one packet per ring, so without the gate the 1072 16KiB
    # adjacency packets would starve the small boot packets for ~40us.
    gate = cpool.tile([1, 4], FP8, tag="gate", name="gate")
    nc.gpsimd.tensor_copy(gate[0:1, 0:2], cf[0:1, CF_W - 2:CF_W])
    nc.gpsimd.tensor_copy(gate[0:1, 2:4], u7[0:1, 1022:1024])
    adjs = resp.tile([128, NB * NQ * 512], FP8, tag="adjs", name="adjs")
    adjs4 = adjs[:].rearrange("p (i q c) -> p i q c", q=NQ, c=512)
    for ib in range(NB):
        # WAW head-write makes the gate binding (Tile would otherwise hoist
        # the dependency-free stripe DMAs ahead of the gate)
        nc.gpsimd.tensor_copy(adjs4[0:1, ib, 0:1, 0:4], gate[0:1, 0:4])
        nc.gpsimd.dma_start(out=adjs4[:, ib, :, :], in_=adjq[ib])

    ident = cpool.tile([128, 128], BF16)
    make_identity(nc, ident[:])

    # HAM warm-up: ~3.4us of continuous PE work on the first mesh quarter so
    # the clock gate opens (4/8 -> 8/8) before the front-end starts; without
    # it the whole front-end + layer-0 region runs at 1.2 GHz.
    warm = ps_misc.tile([128, 512], F32, tag="misc", name="warm")
    for _ in range(8):
        nc.tensor.matmul(warm[:], lhsT=u7[0:36, 0:128], rhs=u7[0:36, 0:512],
                         start=True, stop=True)

    def bcol(col, p0, p1):
        bc0 = CF["biases"][0]
        return cf[p0:p1, bc0 + col:bc0 + col + 1]

    # ---------------- activation tiles ----------------
    xt = actp.tile([128, BC * N], BF16, tag="x")           # [feat, b*N+n]
    fcutT = actp.tile([114, N], BF16, tag="fcutT")         # b0 rows 0:50, b1 64:114
    # rows 42:64 are read by the PE transpose but never written -> must be 0
    # (memset base must be 32-aligned; rows 32:42 are rewritten by drains)
    nc.vector.memset(fcutT[32:64, :], 0.0)
    fcst = actp.tile([128, NQ * MST], FP8, tag="fcst")     # stationary slots
    cvec = actp.tile([128, BC], F32, tag="cvec")
    cvecs = actp.tile([128, BC], F32, tag="cvecs")
    mx = actp.tile([128, NB], F32, tag="mx")
    outsb = actp.tile([128, 1], F32, tag="outsb")
    fcst3 = fcst[:].rearrange("p (q m) -> p q m", m=MST)
    # pein ring buffers: rows 30:32 and 62:64 must stay zero (pw1 has zero
    # rows there) -- zeroed once here, never written in the chunk loop.
    peins = []
    for i in range(4):
        pt = actp.tile([67, 512], BF16, tag=f"pein{i}", name=f"pein{i}")
        nc.vector.memset(pt[:], 0.0)
        peins.append(pt)

    # ---------------- action MLP (tiny, fp32) ----------------
    pa = ps_misc.tile([128, 2], F32, tag="misc")
    nc.tensor.matmul(pa[:], lhsT=vf("aw1a"), rhs=maskT[:], start=True, stop=True)
    a1a = smallp.tile([128, 2], F32, tag="a1a")
    nc.scalar.activation(a1a[:], pa[:], AF.Relu, bias=bcol(0, 0, 128))
    pb = ps_misc.tile([72, 2], F32, tag="misc")
    nc.tensor.matmul(pb[:], lhsT=vf("aw1b"), rhs=maskT[:], start=True, stop=True)
    a1b = smallp.tile([72, 2], F32, tag="a1b")
    nc.scalar.activation(a1b[:], pb[:], AF.Relu, bias=bcol(1, 0, 72))
    pc = ps_misc.tile([100, 2], F32, tag="misc")
    nc.tensor.matmul(pc[:], lhsT=vf("aw2a"), rhs=a1a[:], start=True, stop=False)
    nc.tensor.matmul(pc[:], lhsT=vf("aw2b"), rhs=a1b[:], start=False, stop=True)
    a2 = smallp.tile([100, 2], F32, tag="a2")
    nc.scalar.activation(a2[:], pc[:], AF.Relu, bias=bcol(2, 0, 100))
    pd = ps_misc.tile([100, 2], F32, tag="misc")
    nc.tensor.matmul(pd[:], lhsT=vf("aw3"), rhs=a2[:], start=True, stop=True)
    a3 = smallp.tile([100, 2], F32, tag="a3")
    nc.scalar.activation(a3[:], pd[:], AF.Identity, bias=bcol(3, 0, 100))
    pe_ = ps_misc.tile([128, 2], F32, tag="misc")
    gw0L = [vf("gw0L"), vf("gw0Lp")]
    for b in range(BC):
        nc.tensor.matmul(pe_[:, b:b + 1], lhsT=gw0L[b], rhs=a3[:, b:b + 1],
                         start=True, stop=True)
        nc.scalar.activation(cvec[:, b:b + 1], pe_[:, b:b + 1], AF.Identity,
                             bias=bcol(4 + b, 0, 128))
    nc.scalar.activation(cvecs[:], cvec[:], AF.Identity, scale=SF)

    # one transpose per 128-node block serves both batches; the PSUM->SBUF
    # copies alternate between DVE and scalar.
    def emit_tp(q, mm):
        jc = slice(q * 128, (q + 1) * 128)
        tp = ps_tp.tile([128, 128], BF16, tag="tp", name=f"tp_{q}")
        nc.tensor.transpose(tp[:, 0:mm], fcutT[0:mm, jc], ident[0:mm, 0:mm])
        if q % 2 == 0:
            nc.vector.tensor_copy(fcst3[:, q, 0:mm], tp[:, 0:mm])
        else:
            nc.scalar.activation(fcst3[:, q, 0:mm], tp[:, 0:mm], AF.Identity)

    # ---------------- positional front-end + fused layer-0 features -------
    # Software-pipelined over chunks so the tensor-queue FIFO never stalls on
    # a same-chunk scalar/DVE chain: iteration i runs stage 1 (t2c matmul +
    # fused magic-round range reduction + one Sin per batch) for chunk i,
    # stage 2 (positional MLP + fused layer-0 features) for chunk i-1, the
    # PE transposes for chunk i-2, and weaves in the first four ib-blocks of
    # the layer-0 adjacency matmul as their fcst slots become available.
    # t2c rows are batch-major (b0: sin 0:30 / cos 32:62, b1: +64) with pad
    # rows exactly zero, so ONE [62,512] Sin covers sin+cos per batch.
    w3f = [vb("w3f"), vb("w3fp")]
    t4 = [vb("t4", 64), vb("t4p", 64)]
    pl0 = {}
    for ib in range(2):
        pl0[ib] = ps_left.tile([106, 512], F32, tag="left", name=f"pl0_{ib}")

    def c0_mms(jts):
        for jt in jts:
            lhsT = fcst3[:, 2 * jt:2 * jt + 2, 0:106]
            for ib in range(2):
                nc.tensor.matmul(pl0[ib][:], lhsT=lhsT,
                                 rhs=adjs4[:, ib, 2 * jt:2 * jt + 2, :],
                                 start=(jt == 0), stop=(jt == NQ // 2 - 1),
                                 perf_mode=DR)

    for i in range(NB + 1):
        if i < NB:
            ch = i
            cs = slice(ch * 512, (ch + 1) * 512)
            t2c = ps_misc.tile([126, 512], F32, tag="misc")
            nc.tensor.matmul(t2c[:], lhsT=vf("selfreq"), rhs=u7[0:36, cs],
                             start=True, stop=True)
            r2 = dvep.tile([126, 512], F32, tag="r2")
            nc.vector.tensor_scalar(r2[:], t2c[:], MAGIC, MAGIC,
                                    ALU.add, ALU.subtract)
            dd = dvep.tile([126, 512], F32, tag="dd")
            nc.vector.tensor_sub(dd[:], t2c[:], r2[:])
            for b in range(BC):
                pein = peins[2 * (ch % 2) + b]
                nc.scalar.activation(pein[0:62, :], dd[64 * b:64 * b + 62, :],
                                     AF.Sin, scale=TWO_PI)
                nc.gpsimd.tensor_copy(pein[64:67, :],
                                      u7[32 * b:32 * b + 3, cs])
        if i >= 1:
            ch = i - 1
            cs = slice(ch * 512, (ch + 1) * 512)
            for b in range(BC):
                xs = slice(b * N + ch * 512, b * N + (ch + 1) * 512)
                pein = peins[2 * (ch % 2) + b]
                # h1 = relu(pe_in @ pw1 + pb1)
                ph1 = ps_tp.tile([25, 512], F32, tag="tp")
                nc.tensor.matmul(ph1[:], lhsT=vb("pw1"), rhs=pein[:],
                                 start=True, stop=True)
                h1 = smallp.tile([25, 512], BF16, tag=f"h1{b}")
                if b == 0:
                    nc.scalar.activation(h1[:], ph1[:], AF.Relu,
                                         bias=bcol(6, 0, 25))
                else:
                    nc.vector.tensor_scalar(h1[:], ph1[:], bcol(6, 0, 25),
                                            0.0, ALU.add, ALU.max)
                # h2 = relu(h1 @ pw2 + pb2)
                ph2 = ps_tp.tile([50, 512], F32, tag="tp")
                nc.tensor.matmul(ph2[:], lhsT=vb("pw2"), rhs=h1[:],
                                 start=True, stop=True)
                h2 = smallp.tile([50, 512], BF16, tag=f"h2{b}")
                if b == 0:
                    nc.vector.tensor_scalar(h2[:], ph2[:], bcol(7, 0, 50),
                                            0.0, ALU.add, ALU.max)
                else:
                    nc.scalar.activation(h2[:], ph2[:], AF.Relu,
                                         bias=bcol(7, 0, 50))
                # layer-0 features: f0 = [h2 | onehot] @ [w3fold; t4]+cvec;
                # the K=50 and K=4 matmuls run row-tiled concurrently.
                pf = ps_misc.tile([128, 512], F32, tag="misc")
                nc.tensor.matmul(pf[:], lhsT=w3f[b], rhs=h2[:],
                                 start=True, stop=False)
                nc.tensor.matmul(pf[:], lhsT=t4[b], rhs=onehot[64:68, xs],
                                 start=False, stop=True)
                # full-tile relu: rows overlapping the cut range get garbage
                # and are overwritten by the C drain later.
                if b == 0:
                    nc.scalar.activation(xt[:, xs], pf[:, :], AF.Relu,
                                         bias=cvec[:, 0:1])
                    nc.vector.tensor_scalar(fcutT[0:42, cs], pf[0:42, :],
                                            SF, cvecs[0:42, 0:1],
                                            ALU.mult, ALU.add)
                else:
                    nc.vector.tensor_scalar(xt[:, xs], pf[:, :],
                                            cvec[:, 1:2], 0.0,
                                            ALU.add, ALU.max)
                    nc.scalar.activation(fcutT[64:106, cs], pf[64:106, :],
                                         AF.Identity, bias=cvecs[64:106, 1:2],
                                         scale=SF)
        if i >= 2:
            for q in range(4 * (i - 2), 4 * (i - 1)):
                emit_tp(q, 106)
        if i >= 4:
            c0_mms(range(2 * (i - 4), 2 * (i - 4) + 2))
    for q in range(4 * (NB - 1), NQ):
        emit_tp(q, 106)
    c0_mms(range(10, NQ // 2))

    # ---------------- GCN layers ----------------
    def drain_c(li, ib, pl, last, split=False):
        if split and not last:
            # half-width drains on both engines: ~halves the drain backlog
            # that gates the next layer's phase A at the layer boundary
            for b, p0, p1, xoff in ((0, 0, 42, 0), (1, 64, 106, N)):
                for h in range(2):
                    c0 = ib * 512 + h * 256
                    eng_v = (b + h) % 2 == 0
                    if eng_v:
                        nc.vector.tensor_scalar(
                            xt[p0:p1, xoff + c0:xoff + c0 + 256],
                            pl[p0:p1, h * 256:h * 256 + 256],
                            bcol(8 + li, p0, p1), 0.0, ALU.add, ALU.max)
                    else:
                        nc.scalar.activation(
                            xt[p0:p1, xoff + c0:xoff + c0 + 256],
                            pl[p0:p1, h * 256:h * 256 + 256],
                            AF.Relu, bias=bcol(8 + li, p0, p1))
            return
        if not last:
            # xt keeps the (SA*SF)-scaled cut values; the next-layer gw cut
            # rows are pre-divided on the host, and the gb cut biases are
            # pre-multiplied (cols 8-10).  Drains alternate engines.
            if ib % 2 == 0:
                nc.vector.tensor_scalar(
                    xt[0:42, ib * 512:(ib + 1) * 512],
                    pl[0:42, :], bcol(8 + li, 0, 42), 0.0,
                    ALU.add, ALU.max)
                nc.scalar.activation(
                    xt[64:106, N + ib * 512:N + (ib + 1) * 512],
                    pl[64:106, :], AF.Relu, bias=bcol(8 + li, 64, 106))
            else:
                nc.scalar.activation(
                    xt[0:42, ib * 512:(ib + 1) * 512],
                    pl[0:42, :], AF.Relu, bias=bcol(8 + li, 0, 42))
                nc.vector.tensor_scalar(
                    xt[64:106, N + ib * 512:N + (ib + 1) * 512],
                    pl[64:106, :], bcol(8 + li, 64, 106), 0.0,
                    ALU.add, ALU.max)
        else:
            # fused max-reduce over all 114 rows (rows 50:64 are zero)
            nc.vector.tensor_reduce(
                mx[0:114, ib:ib + 1], pl[0:114, :],
                mybir.AxisListType.X, mybir.AluOpType.max)

    gws = {1: [vb("gw1"), vb("gw1p")], 2: [vb("gw2"), vb("gw2p")],
           3: [vb("gw3"), vb("gw3p")]}

    def emit_a(li, ch):
        """Phase A of layer li for one chunk: f = x @ gw, drains to xt/fcutT."""
        last = li == 3
        cs = slice(ch * 512, (ch + 1) * 512)
        if last:
            pf = ps_misc.tile([128, 512], F32, tag="misc")
            for b in range(BC):
                xs = slice(b * N + ch * 512, b * N + (ch + 1) * 512)
                nc.tensor.matmul(pf[64 * b:64 * b + 50, :], lhsT=gws[3][b],
                                 rhs=xt[:, xs], start=True, stop=True)
            nc.scalar.activation(fcutT[0:50, cs], pf[0:50, :],
                                 AF.Identity, scale=SF)
            nc.vector.tensor_scalar_mul(fcutT[64:114, cs], pf[64:114, :], SF)
        else:
            for b in range(BC):
                xs = slice(b * N + ch * 512, b * N + (ch + 1) * 512)
                pf = ps_misc.tile([128, 512], F32, tag="misc")
                nc.tensor.matmul(pf[:], lhsT=gws[li][b], rhs=xt[:, xs],
                                 start=True, stop=True)
                # full-tile relu; cut rows get garbage and are rewritten by
                # the next C drain.
                if b == 0:
                    nc.scalar.activation(xt[:, xs], pf[:, :], AF.Relu)
                    nc.vector.tensor_scalar_mul(fcutT[0:42, cs],
                                                pf[0:42, :], SF)
                else:
                    nc.vector.tensor_scalar_max(xt[:, xs], pf[:, :], 0.0)
                    nc.scalar.activation(fcutT[64:106, cs], pf[64:106, :],
                                         AF.Identity, scale=SF)

    # Layer loop: phase A (f = x @ gw) with the next transposes woven one
    # chunk behind its drains, then phase C in ib-groups of 2 with drains
    # alternating engines.  Layer 0's ib 0-1 were woven into the front-end.
    for li in range(4):
        last = li == 3
        mm = 114 if last else 106
        if li > 0:
            for ch in range(NB):
                if ch >= 1:
                    for q in range(4 * (ch - 1), 4 * ch):
                        emit_tp(q, mm)
                emit_a(li, ch)
            for q in range(4 * (NB - 1), NQ):
                emit_tp(q, mm)
        if li == 0:
            for ib in range(2):
                drain_c(0, ib, pl0[ib], last)
            groups = [(2, 4), (4, 6), (6, 8)]
        else:
            groups = [(0, 2), (2, 4), (4, 6), (6, 8)]
        for g0, g1 in groups:
            pls = {}
            for ib in range(g0, g1):
                pls[ib] = ps_left.tile([mm, 512], F32, tag="left",
                                       name=f"pl{li}_{ib}")
            for jt in range(NQ // 2):
                lhsT = fcst3[:, 2 * jt:2 * jt + 2, 0:mm]
                for ib in range(g0, g1):
                    rhs = adjs4[:, ib, 2 * jt:2 * jt + 2, :]
                    nc.tensor.matmul(pls[ib][:], lhsT=lhsT, rhs=rhs,
                                     start=(jt == 0), stop=(jt == NQ // 2 - 1),
                                     perf_mode=DR)
            for ib in range(g0, g1):
                drain_c(li, ib, pls[ib], last, split=(g0 == 6))

    # ---------------- final max + bias + output ----------------
    mxr = smallp.tile([128, 1], F32, tag="mxr")
    nc.vector.tensor_reduce(mxr[:], mx[:], mybir.AxisListType.X,
                            mybir.AluOpType.max)
    nc.scalar.activation(outsb[:], mxr[:], AF.Identity, bias=bcol(11, 0, 128),
                         scale=INV)
    nc.sync.dma_start(out=outd[:], in_=outsb[:])
    ctx.close()


# ---------------------------------------------------------------------------
# host side
# ---------------------------------------------------------------------------

# batch-1 feature permutation: rows 0:64 <- features 42:106,
# rows 64:106 <- features 0:42 (the cut), rows 106:128 <- features 106:128
PI = np.concatenate([np.arange(42, 106), np.arange(0, 42),
                     np.arange(106, 128)]).astype(np.int64)


def _prep_shared(inp):
    """Host preprocessing shared across cores (weights + adj)."""
    f32 = np.float32
    adjT = np.ascontiguousarray(inp["adj"].astype(f32).T * f32(SA))
    # [src, dst] -> [ib, p, q, c] with src = q*128+p, dst = ib*512+c
    adjq = np.ascontiguousarray(
        adjT.reshape(NQ, 128, NB, 512).transpose(2, 1, 0, 3)).astype(E4)

    gw0 = inp["gw0"].astype(f32)
    w3fold = (inp["pw3"].astype(f32) @ gw0[100:200])
    t4 = (inp["emb"].astype(f32) @ gw0[200:300])
    pb3f = (inp["pb3"].astype(f32) @ gw0[100:200]).astype(f32)
    gw0L = np.ascontiguousarray(gw0[:100])

    # pe_in row permutation: ours = [sin(f,c) x30 | cos(f,c) x30 | mesh x3]
    pw1f = inp["pw1"].astype(f32)
    pw1p_ = np.zeros((67, 25), f32)
    for k in range(30):
        f, c = divmod(k, 3)
        pw1p_[k] = pw1f[f * 6 + c]          # sin rows
        pw1p_[32 + k] = pw1f[f * 6 + 3 + c]  # cos rows
    pw1p_[64:67] = pw1f[60:63]

    freqs = np.asarray([np.pi] + [2.0 * np.pi * i for i in range(1, 10)], f32)
    freq2 = np.repeat(freqs, 3) / (2.0 * np.pi)   # [30]
    # selfreq [36, 126]: cols 0:30 b0 sin, 32:62 b1 sin, 64:94 b0 cos,
    # 96:126 b1 cos (cos args get +0.25 via the ones row 4); mesh input rows
    # 0:3 b0 xyz, 32:35 b1 xyz (32-aligned partition bases everywhere)
    self7 = np.zeros((36, 126), f32)
    for b in range(2):
        for k in range(30):
            self7[32 * b + k % 3, 64 * b + k] = freq2[k]
            self7[32 * b + k % 3, 64 * b + 32 + k] = freq2[k]
            self7[4, 64 * b + 32 + k] = 0.25

    # xt carries the adjacency-matmul output still scaled by SA*SF; undo the
    # scale by pre-dividing the gw rows that consume cut features (b0 rows
    # 0:42, permuted-b1 rows 64:106) and pre-multiplying the gb cut biases.
    gw1 = inp["gw1"].astype(f32)
    gw2 = inp["gw2"].astype(f32)
    gw3 = inp["gw3"].astype(f32)
    gw1p = gw1[np.ix_(PI, PI)].copy()
    gw2p = gw2[np.ix_(PI, PI)].copy()
    gw3p = gw3[PI, :].copy()
    for g in (gw1, gw2, gw3):
        g[0:42] *= f32(INV)
    for g in (gw1p, gw2p, gw3p):
        g[64:106] *= f32(INV)

    biasd = np.zeros((128, 12), f32)
    biasd[0:128, 0] = inp["ab1"][:128]
    biasd[0:72, 1] = inp["ab1"][128:200]
    biasd[0:100, 2] = inp["ab2"]
    biasd[0:100, 3] = inp["ab3"]
    biasd[0:128, 4] = pb3f
    biasd[0:128, 5] = pb3f[PI]
    biasd[0:25, 6] = inp["pb1"].astype(f32)
    biasd[0:50, 7] = inp["pb2"].astype(f32)
    for li in range(3):
        gb = inp[f"gb{li}"].astype(f32) * f32(SA * SF)
        biasd[0:42, 8 + li] = gb[:42]
        biasd[64:106, 8 + li] = gb[:42]
    gb3 = inp["gb3"].astype(f32)
    biasd[0:50, 11] = gb3
    biasd[64:114, 11] = gb3

    cf32 = np.zeros((128, CF_W), f32)
    for key, arr in (("aw1a", inp["aw1"].astype(f32)[:, :128]),
                     ("aw1b", inp["aw1"].astype(f32)[:, 128:200]),
                     ("aw2a", inp["aw2"].astype(f32)[:128]),
                     ("aw2b", inp["aw2"].astype(f32)[128:200]),
                     ("aw3", inp["aw3"].astype(f32)),
                     ("gw0L", gw0L), ("gw0Lp", gw0L[:, PI]),
                     ("biases", biasd), ("selfreq", self7)):
        c0, w, r = CF[key]
        assert arr.shape == (r, w), (key, arr.shape)
        cf32[0:r, c0:c0 + w] = arr

    cb16 = np.zeros((128, CB_W), BF)
    for key, arr, r0 in (("pw1", pw1p_, 0), ("pw2", inp["pw2"].astype(f32), 0),
                         ("w3f", w3fold, 0), ("w3fp", w3fold[:, PI], 0),
                         ("gw1", gw1, 0), ("gw1p", gw1p, 0),
                         ("gw2", gw2, 0), ("gw2p", gw2p, 0),
                         ("gw3", gw3, 0), ("gw3p", gw3p, 0),
                         ("t4", t4, 64), ("t4p", t4[:, PI], 64)):
        c0, w, r = CB[key]
        assert arr.shape == (r, w), (key, arr.shape)
        cb16[r0:r0 + r, c0:c0 + w] = arr.astype(BF)

    return {"adjq": adjq, "cf32d": cf32, "cb16d": cb16}


def _prep_core(inp, shared, core):
    bs = slice(core * BC, (core + 1) * BC)
    f32 = np.float32
    mesh = inp["mesh"].astype(f32)[bs]                       # [2, N, 3]
    mesh7 = np.zeros((36, N), f32)                           # b0 xyz @0, b1 @32
    mesh7[0:3] = mesh[0].T
    mesh7[32:35] = mesh[1].T
    mesh7[4] = 1.0                                           # ones row
    mi = inp["mask_idx"][bs]                                 # [2, N] int32
    onehot = (mi[:, None, :] == np.arange(4, dtype=mi.dtype)[None, :, None])
    onehot = np.ascontiguousarray(
        onehot.transpose(1, 0, 2).reshape(4, BC * N)).astype(BF)
    maskT = np.ascontiguousarray(inp["mask"].astype(f32)[bs].T)  # [50, 2]
    m = dict(shared)
    m["mesh7d"] = mesh7
    m["onehotd"] = onehot
    m["maskTd"] = maskT
    return m


_CACHED = {}


def kernel(**inputs) -> np.ndarray:
    if "nc" not in _CACHED:
        _CACHED["nc"] = build_bass()
    nc = _CACHED["nc"]
    shared = _prep_shared(inputs)
    in_maps = [_prep_core(inputs, shared, c) for c in range(NCORES)]
    res = run_bass_kernel_spmd(nc, in_maps, list(range(NCORES)), **run_kwargs)
    out = np.empty((B, 50), np.float32)
    for c in range(NCORES):
        o = res.results[c]["outd"][:, 0]
        out[2 * c] = o[0:50]
        out[2 * c + 1] = o[64:114]
    _CACHED["last_results"] = res
    return out
